# revision 1
# baseline (speedup 1.0000x reference)
"""Trainium2 Bass kernel for single-head causal attention.

Problem: x[B=4,T=2048,C=1024] -> q,k,v = x@Wq/Wk/Wv [T,64] -> causal softmax(q k^T/sqrt(C)) @ v.

Sharding: 8 cores = 4 batches x 2 query-halves (sequence-parallel queries,
replicated weights). Each core computes K/V projections for the full
sequence and attention for its 1024 queries.

SPMD-uniform trick: the time axis of each core's x^T copy is permuted so the
core's OWN query half comes first (columns 0-1023), the other half after.
Then the causal block structure is identical on every core:
  - k-tiles j=0..7  (own half): lower-triangular blocks, diagonal gets a
    constant 128x128 triangular mask; blocks above the diagonal are skipped.
  - k-tiles j=8..15 (other half): full rectangle whose validity differs only
    by DATA: an exp-bias "gate" per core (0.0 => keep, -60 => exp ~ 0).
Softmax normalization is fused into the AV matmul by appending a ones column
to V (output row 64 = sum of exp); division happens host-side on gather.

Device layout: S^T = k_tile^T-stationary x q^T-moving so the softmax free
dim is q and P^T feeds AV directly with V-natural stationary (V transposed
on-device via the DMA xbar, bf16).
"""

import numpy as np
import ml_dtypes

B, T, C, H = 4, 2048, 1024, 64
TQ = 1024          # queries per core
NT = 2048          # kv length per core
NCH = C // 128     # 8 contraction chunks
NKT = NT // 128    # 16 k-tiles
SCALE = 1.0 / 32.0  # 1/sqrt(C)
VSTRIDE = 80       # bf16 cols per v' tile slot (64 v + 1 ones + pad, 32B-aligned)

_prog_cache = {}


def _build_program():
    import concourse.mybir as mybir
    from concourse import bacc
    from concourse.tile import TileContext

    fp32 = mybir.dt.float32
    bf16 = mybir.dt.bfloat16
    Exp = mybir.ActivationFunctionType.Exp

    nc = bacc.Bacc("TRN2", target_bir_lowering=False, debug=False)

    xt_d = nc.dram_tensor("xt", [C, NT], bf16, kind="ExternalInput")
    wqk_d = nc.dram_tensor("wqk", [C, 128], bf16, kind="ExternalInput")
    wv_d = nc.dram_tensor("wv", [C, H], bf16, kind="ExternalInput")
    gate_d = nc.dram_tensor("gate", [128, 1], fp32, kind="ExternalInput")
    tri_d = nc.dram_tensor("tri", [128, 128], bf16, kind="ExternalInput")
    idn_d = nc.dram_tensor("idn", [64, 64], bf16, kind="ExternalInput")
    out_d = nc.dram_tensor("outT", [H + 1, TQ], fp32, kind="ExternalOutput")

    with TileContext(nc) as tc:
        with (
            tc.tile_pool(name="xtp", bufs=1) as xt_pool,
            tc.tile_pool(name="cst", bufs=1) as cst,
            tc.tile_pool(name="prj", bufs=1) as prj,
            tc.tile_pool(name="ptp", bufs=8) as ptp,
            tc.tile_pool(name="psA", bufs=1, space="PSUM") as psA,
            tc.tile_pool(name="psB", bufs=1, space="PSUM") as psB,
            tc.tile_pool(name="psS", bufs=2, space="PSUM") as psS,
            tc.tile_pool(name="psO", bufs=2, space="PSUM") as psO,
        ):
            # constants / weights
            wqk_sb = cst.tile([128, NCH, 128], bf16, tag="wqk")
            nc.sync.dma_start(out=wqk_sb[:], in_=wqk_d.rearrange("(o p) m -> p o m", p=128))
            wv_sb = cst.tile([128, NCH, H], bf16, tag="wv")
            nc.sync.dma_start(out=wv_sb[:], in_=wv_d.rearrange("(o p) m -> p o m", p=128))
            gate_sb = cst.tile([128, 1], fp32, tag="gate")
            nc.sync.dma_start(out=gate_sb[:], in_=gate_d[:])
            tri_sb = cst.tile([128, 128], bf16, tag="tri")
            nc.sync.dma_start(out=tri_sb[:], in_=tri_d[:])
            idn_sb = cst.tile([64, 64], bf16, tag="idn")
            nc.sync.dma_start(out=idn_sb[:], in_=idn_d[:])

            # x^T chunks (C on partitions)
            xt_sb = []
            for c in range(NCH):
                t = xt_pool.tile([128, NT], bf16, tag=f"xt{c}")
                nc.sync.dma_start(out=t[:], in_=xt_d[c * 128:(c + 1) * 128, :])
                xt_sb.append(t)

            # persistent projection outputs
            qT_sb = prj.tile([64, TQ], fp32, tag="qT")
            kT_sb = prj.tile([64, NT], fp32, tag="kT")
            vT_sb = prj.tile([64, NT], bf16, tag="vT")
            vp_sb = prj.tile([128, NKT * VSTRIDE], bf16, tag="vp")
            o_sb = prj.tile([H + 1, TQ], fp32, tag="osb")

            # ones column (col 64 of each VSTRIDE block) for the l-row trick
            nc.vector.memset(
                vp_sb.rearrange("p (t c) -> p t c", c=VSTRIDE)[:, :, 64:65], 1.0
            )

            # PE warmup during the initial x^T DMA (copied to a dummy spot
            # so the verifier sees a reader)
            wq_flat = wqk_sb.rearrange("p o m -> p (o m)")
            scratch = psS.tile([128, 512], fp32, tag="s")
            for _ in range(8):
                nc.tensor.matmul(
                    scratch[:], wqk_sb[:, 0, :], wq_flat[:, 0:512],
                    start=True, stop=True,
                )
            nc.vector.tensor_copy(out=vp_sb[:, 0:64], in_=scratch[:, 0:64])

            # AV accumulators (allocated later, after the v transposes
            # borrow the psO slots)
            o_ps = []

            def emit_quarter(tq):
                """Project 512 time columns: q^T/k^T (packed) and v^T."""
                sl = slice(tq * 512, (tq + 1) * 512)
                qk_ps = psA.tile([128, 512], fp32, tag="qk")
                v_ps = psB.tile([64, 512], fp32, tag="pv")
                for c in range(NCH):
                    nc.tensor.matmul(
                        qk_ps[:], wqk_sb[:, c, :], xt_sb[c][:, sl],
                        start=(c == 0), stop=(c == NCH - 1),
                    )
                for c in range(NCH):
                    nc.tensor.matmul(
                        v_ps[:], wv_sb[:, c, :], xt_sb[c][:, sl],
                        start=(c == 0), stop=(c == NCH - 1),
                    )
                if tq < TQ // 512:
                    nc.vector.tensor_copy(out=qT_sb[:, sl], in_=qk_ps[0:64, :])
                nc.vector.tensor_copy(out=kT_sb[:, sl], in_=qk_ps[64:128, :])
                nc.vector.tensor_copy(out=vT_sb[:, sl], in_=v_ps[:])

            pt_tiles = {}

            def emit_S(j):
                """One k-tile: S^T matmul, exp (with gate bias), diagonal mask."""
                a0 = 128 * j if j < 8 else 0
                s_ps = psS.tile([128, 1024], fp32, tag="s")
                for b in (0, 1):
                    lo, hi = max(a0, 512 * b), 512 * (b + 1)
                    if lo < hi:
                        nc.tensor.matmul(
                            s_ps[:, lo:hi],
                            kT_sb[:, 128 * j: 128 * (j + 1)],
                            qT_sb[:, lo:hi],
                            start=True, stop=True,
                        )
                pt = ptp.tile([128, 1024], bf16, tag="pt")
                bias = gate_sb[:, 0:1] if j >= 8 else 0.0
                nc.scalar.activation(
                    pt[:, a0:1024], s_ps[:, a0:1024], Exp, bias=bias, scale=SCALE
                )
                if j < 8:
                    nc.vector.tensor_mul(
                        pt[:, 128 * j: 128 * (j + 1)],
                        pt[:, 128 * j: 128 * (j + 1)],
                        tri_sb[:],
                    )
                pt_tiles[j] = pt

            def emit_AV(j):
                a0 = 128 * j if j < 8 else 0
                pt = pt_tiles.pop(j)
                for b in (0, 1):
                    lo, hi = max(a0, 512 * b), 512 * (b + 1)
                    if lo < hi:
                        nc.tensor.matmul(
                            o_ps[b][:, lo - 512 * b: hi - 512 * b],
                            vp_sb[:, VSTRIDE * j: VSTRIDE * j + 65],
                            pt[:, lo:hi],
                            start=(j == 0), stop=(j == NKT - 1),
                            skip_group_check=True,
                        )

            emit_quarter(0)
            emit_quarter(1)
            emit_S(0)
            emit_S(1)
            emit_quarter(2)
            emit_S(2)
            emit_S(3)
            emit_quarter(3)
            emit_S(4)
            emit_S(5)

            # v^T -> v-natural via DMA xbar transpose (bf16)
            for t in range(NKT):
                nc.sync.dma_start(
                    out=vp_sb[:, VSTRIDE * t: VSTRIDE * t + 64],
                    in_=vT_sb[:, 128 * t: 128 * (t + 1)],
                    transpose=True,
                )

            o_ps0 = psO.tile([H + 1, 512], fp32, tag="o")
            o_ps1 = psO.tile([H + 1, 512], fp32, tag="o")
            o_ps.extend([o_ps0, o_ps1])

            for j in range(6):
                emit_AV(j)
            for j in range(6, NKT):
                emit_S(j)
                emit_AV(j)

            for b in (0, 1):
                nc.vector.tensor_copy(
                    out=o_sb[:, 512 * b: 512 * (b + 1)], in_=o_ps[b][:]
                )
            nc.sync.dma_start(out=out_d[:], in_=o_sb[:])

    nc.finalize()
    return nc


def _get_program():
    if "nc" not in _prog_cache:
        _prog_cache["nc"] = _build_program()
    return _prog_cache["nc"]


def make_in_maps(x, Wq, Wk, Wv):
    bf16 = ml_dtypes.bfloat16
    wqk = np.concatenate([Wq, Wk], axis=1).astype(bf16)  # [C, 128]
    wv = np.ascontiguousarray(Wv.astype(bf16))
    tri = np.triu(np.ones((128, 128), np.float32)).astype(bf16)  # tri[k,q]=1 iff q>=k
    idn = np.eye(64, dtype=np.float32).astype(bf16)
    in_maps = []
    for core in range(8):
        b, r = core // 2, core % 2
        qs = r * TQ
        other = (1 - r) * TQ
        xb = np.asarray(x[b])
        xt = np.concatenate([xb[qs:qs + TQ], xb[other:other + TQ]], axis=0).T
        gate = np.full((128, 1), 0.0 if r == 1 else -60.0, np.float32)
        in_maps.append({
            "xt": np.ascontiguousarray(xt).astype(bf16),
            "wqk": wqk,
            "wv": wv,
            "gate": gate,
            "tri": tri,
            "idn": idn,
        })
    return in_maps


def postprocess(results):
    out = np.empty((B, T, H), np.float32)
    for core in range(8):
        b, r = core // 2, core % 2
        qs = r * TQ
        oT = results[core]["outT"]  # [65, 1024]
        out[b, qs:qs + TQ] = (oT[:H] / oT[H:H + 1]).T
    return out


def kernel(x, mask, Wq, Wk, Wv, _trace=False, _tracedir=None):
    from concourse import bass_utils

    nc = _get_program()
    in_maps = make_in_maps(np.asarray(x, np.float32), np.asarray(Wq, np.float32),
                           np.asarray(Wk, np.float32), np.asarray(Wv, np.float32))
    res = bass_utils.run_bass_kernel_spmd(
        nc, in_maps, core_ids=list(range(8)),
        trace=_trace, tmpdir=_tracedir,
    )
    out = postprocess(res.results)
    if _trace:
        return out, res
    return out



# revision 6
# speedup vs baseline: 1.8841x; 1.8841x over previous
"""Trainium2 Bass kernel for single-head causal attention.

Problem: x[B=4,T=2048,C=1024] -> q,k,v = x@Wq/Wk/Wv [T,64] -> causal softmax(q k^T/sqrt(C)) @ v.

Sharding: 8 cores = 4 batches x 2 query-interleavings. Core r of a batch owns
the 8 INTERLEAVED query blocks g === r (mod 2) (128 rows each), which balances
causal work across the pair (each core gets ~half the attention area).

SPMD-uniform trick: the time axis of each core's x^T copy is permuted so the
core's OWN blocks come first (columns 0-1023, ascending global index), the
other 8 blocks after (ascending). Then the block-causal structure is
identical on every core:
  - k-slot t=0..7  (own blocks, global 2t+r): S over q cols [128t, 1024);
    the leading 128x128 block is the diagonal -> multiplied by a constant
    triangular mask.
  - k-slot t=8..15 (other blocks, global 2(t-8)+1-r): S over q cols
    [128(t-8), 1024); the leading block differs only by DATA: an all-ones
    (r=1: k-block just below the diagonal -> keep) or all-zeros (r=0:
    k-block just above -> drop) multiplier per core.
Softmax normalization is fused into the AV matmul by appending a ones column
to V (output row 64 = sum of exp); division happens host-side on gather.

All matmuls stream bf16 (fp32 matmul is 4 cycles/row). Projections run
chunk-major so they pipeline with the x^T DMA; the own half is loaded and
projected first so S/exp start as early as possible. V is transposed to
natural layout with two batched DMA-xbar transposes.
"""

import numpy as np
import ml_dtypes

B, T, C, H = 4, 2048, 1024, 64
TQ = 1024          # queries per core
NT = 2048          # kv length per core
NCH = C // 128     # 8 contraction chunks
NKT = NT // 128    # 16 k-slots
SCALE = 1.0 / 32.0  # 1/sqrt(C)
VSTRIDE = 80       # bf16 cols per v' slot (64 v + 1 ones + pad, 32B-aligned)

_prog_cache = {}


def _build_program():
    import concourse.mybir as mybir
    from concourse import bacc
    from concourse.tile import TileContext

    fp32 = mybir.dt.float32
    bf16 = mybir.dt.bfloat16
    Exp = mybir.ActivationFunctionType.Exp

    nc = bacc.Bacc("TRN2", target_bir_lowering=False, debug=False)

    xt_d = nc.dram_tensor("xt", [C, NT], bf16, kind="ExternalInput")
    wcat_d = nc.dram_tensor("wcat", [C, 192], bf16, kind="ExternalInput")
    trig_d = nc.dram_tensor("trig", [128, 256], bf16, kind="ExternalInput")
    out_d = nc.dram_tensor("outT", [H + 1, TQ], fp32, kind="ExternalOutput")

    with TileContext(nc) as tc:
        with (
            tc.tile_pool(name="xtp", bufs=1) as xt_pool,
            tc.tile_pool(name="cst", bufs=1) as cst,
            tc.tile_pool(name="prj", bufs=1) as prj,
            tc.tile_pool(name="ptp", bufs=6) as ptp,
            tc.tile_pool(name="pqk", bufs=2, space="PSUM") as pqk,
            tc.tile_pool(name="pvo", bufs=2, space="PSUM") as pvo,
            tc.tile_pool(name="pss", bufs=2, space="PSUM") as pss,
        ):
            # const DMAs on the scalar queue (idle until exp starts)
            wcat_sb = cst.tile([128, NCH, 192], bf16, tag="wcat")
            nc.scalar.dma_start(out=wcat_sb[:], in_=wcat_d.rearrange("(o p) m -> p o m", p=128))
            trig_sb = cst.tile([128, 256], bf16, tag="trig")
            nc.scalar.dma_start(out=trig_sb[:], in_=trig_d[:])

            # x^T halves, 2 chunks per DMA (cols of pair tile: chunk 2p | 2p+1)
            xo_sb, xr_sb = [], []
            for half, lst in ((0, xo_sb), (1, xr_sb)):
                for p in range(4):
                    t = xt_pool.tile([128, 2048], bf16, tag=f"x{half}{p}")
                    nc.sync.dma_start(
                        out=t.rearrange("p (o m) -> p o m", o=2),
                        in_=xt_d[256 * p: 256 * (p + 1),
                                 1024 * half: 1024 * (half + 1)]
                        .rearrange("(o p) m -> p o m", p=128),
                    )
                    lst.append(t)

            def xch(c):
                """rhs AP for contraction chunk c: [128, 1024] time cols."""
                src = xo_sb if c < 8 else xr_sb
                cc = c % 8
                return src[cc // 2][:, 1024 * (cc % 2): 1024 * (cc % 2) + 1024]

            # persistent projection outputs (bf16 so S/AV stream at full rate)
            qT_sb = prj.tile([64, TQ], bf16, tag="qT")
            kT_sb = prj.tile([64, NT], bf16, tag="kT")
            vT_sb = prj.tile([64, NT], bf16, tag="vT")
            vp_sb = prj.tile([128, NKT * VSTRIDE], bf16, tag="vp")
            o_sb = prj.tile([H + 1, TQ], fp32, tag="osb")

            vp3 = vp_sb.rearrange("p (t c) -> p t c", c=VSTRIDE)
            nc.gpsimd.memset(vp3[:, :, 64:65], 1.0)

            # ---- pass 1: own half (q, k, v), chunk-major ----
            qk_ps = [pqk.tile([128, 512], fp32, tag="qk", name=f"qk{h}")
                     for h in range(2)]
            v_ps = [pvo.tile([64, 512], fp32, tag="vo", name=f"pv{h}",
                             padded_shape=[128, 512])
                    for h in range(2)]
            for c in range(NCH):
                for h in range(2):
                    nc.tensor.matmul(
                        qk_ps[h][:], wcat_sb[:, c, 0:128],
                        xch(c)[:, 512 * h: 512 * (h + 1)],
                        start=(c == 0), stop=(c == NCH - 1),
                    )
                for h in range(2):
                    nc.tensor.matmul(
                        v_ps[h][:], wcat_sb[:, c, 128:192],
                        xch(c)[:, 512 * h: 512 * (h + 1)],
                        start=(c == 0), stop=(c == NCH - 1),
                    )
            for h in range(2):
                sl = slice(512 * h, 512 * (h + 1))
                nc.vector.tensor_copy(out=qT_sb[:, sl], in_=qk_ps[h][0:64, :])
                nc.vector.tensor_copy(out=kT_sb[:, sl], in_=qk_ps[h][64:128, :])
                nc.vector.tensor_copy(out=vT_sb[:, sl], in_=v_ps[h][:])

            # own-half v -> natural layout (batched xbar transpose)
            nc.sync.dma_start_transpose(
                out=vp3[:, 0:8, 0:64], in_=vT_sb[:, 0:1024])

            # ---- pass 2: other half (k, v packed), chunk-major ----
            kv_ps = [pqk.tile([128, 512], fp32, tag="qk", name=f"kv{h}")
                     for h in range(2)]

            def emit_pass2(cs):
                for c in cs:
                    for h in range(2):
                        nc.tensor.matmul(
                            kv_ps[h][:], wcat_sb[:, c, 64:192],
                            xch(8 + c)[:, 512 * h: 512 * (h + 1)],
                            start=(c == 0), stop=(c == NCH - 1),
                        )

            def emit_pass2_tail():
                for h in range(2):
                    sl = slice(1024 + 512 * h, 1536 + 512 * h)
                    nc.vector.tensor_copy(out=kT_sb[:, sl], in_=kv_ps[h][0:64, :])
                    nc.vector.tensor_copy(out=vT_sb[:, sl], in_=kv_ps[h][64:128, :])
                nc.sync.dma_start_transpose(
                    out=vp3[:, 8:16, 0:64], in_=vT_sb[:, 1024:2048])

            # ---- S / exp / AV phase ----
            o_ps = [pvo.tile([H + 1, 512], fp32, tag="vo", name=f"o{h}",
                             padded_shape=[128, 512])
                    for h in range(2)]
            pt_tiles = {}

            def emit_S(t):
                a0 = 128 * (t % 8)
                s = pss.tile([128, 1024], fp32, tag="s")
                if a0 < 512:
                    nc.tensor.matmul(
                        s[:, a0:512], kT_sb[:, 128 * t: 128 * (t + 1)],
                        qT_sb[:, a0:512], start=True, stop=True,
                    )
                nc.tensor.matmul(
                    s[:, 512:1024], kT_sb[:, 128 * t: 128 * (t + 1)],
                    qT_sb[:, 512:1024], start=True, stop=True,
                )
                pt = ptp.tile([128, 1024], bf16, tag="pt")
                nc.scalar.activation(pt[:, a0:1024], s[:, a0:1024], Exp, scale=SCALE)
                msk = trig_sb[:, 0:128] if t < 8 else trig_sb[:, 128:256]
                nc.vector.tensor_mul(
                    pt[:, a0:a0 + 128], pt[:, a0:a0 + 128], msk)
                pt_tiles[t] = pt

            def emit_AV(t):
                a0 = 128 * (t % 8)
                pt = pt_tiles.pop(t)
                if a0 < 512:
                    nc.tensor.matmul(
                        o_ps[0][:, a0:512],
                        vp3[:, t, 0:65], pt[:, a0:512],
                        start=(t == 0), stop=(t == 11),
                        skip_group_check=True,
                    )
                b0 = max(a0, 512)
                nc.tensor.matmul(
                    o_ps[1][:, b0 - 512: 512],
                    vp3[:, t, 0:65], pt[:, b0:1024],
                    start=(t == 0), stop=(t == NKT - 1),
                    skip_group_check=True,
                )

            # interleave: pass2 tracks its DMA; S fills the PE between;
            # AV lags S by 3 so exp+mask have time to land.
            emit_pass2(range(0, 4))
            emit_S(0)
            emit_S(1)
            emit_S(2)
            emit_S(3)
            emit_AV(0)
            emit_pass2(range(4, 8))
            emit_pass2_tail()
            emit_S(4)
            emit_AV(1)
            emit_S(5)
            emit_AV(2)
            emit_S(6)
            emit_AV(3)
            emit_S(7)
            emit_AV(4)
            for t in range(8, NKT):
                emit_S(t)
                emit_AV(t - 3)
            for t in range(NKT - 3, NKT):
                emit_AV(t)

            for h in (0, 1):
                nc.vector.tensor_copy(
                    out=o_sb[:, 512 * h: 512 * (h + 1)], in_=o_ps[h][:]
                )
            nc.sync.dma_start(out=out_d[:], in_=o_sb[:])

    nc.finalize()
    return nc


def _get_program():
    if "nc" not in _prog_cache:
        _prog_cache["nc"] = _build_program()
    return _prog_cache["nc"]


def make_in_maps(x, Wq, Wk, Wv):
    bf16 = ml_dtypes.bfloat16
    wcat = np.concatenate([Wq, Wk, Wv], axis=1).astype(bf16)  # [C, 192]
    tri = np.triu(np.ones((128, 128), np.float32))  # tri[k,q]=1 iff q>=k
    in_maps = []
    for core in range(8):
        b, r = core // 2, core % 2
        xb = np.asarray(x[b]).reshape(16, 128, C)
        own = xb[r::2].reshape(TQ, C)
        other = xb[1 - r::2].reshape(TQ, C)
        xt = np.concatenate([own, other], axis=0).T
        gate = np.full((128, 128), float(r), np.float32)
        trig = np.concatenate([tri, gate], axis=1).astype(bf16)  # [128, 256]
        in_maps.append({
            "xt": np.ascontiguousarray(xt).astype(bf16),
            "wcat": wcat,
            "trig": trig,
        })
    return in_maps


def postprocess(results):
    out = np.empty((B, T, H), np.float32)
    for core in range(8):
        b, r = core // 2, core % 2
        oT = results[core]["outT"]  # [65, 1024]
        o = (oT[:H] / oT[H:H + 1]).T  # [1024, 64] local q order
        for i in range(8):
            g = 2 * i + r
            out[b, 128 * g: 128 * (g + 1)] = o[128 * i: 128 * (i + 1)]
    return out


def kernel(x, mask, Wq, Wk, Wv, _trace=False, _tracedir=None):
    from concourse import bass_utils

    nc = _get_program()
    in_maps = make_in_maps(np.asarray(x, np.float32), np.asarray(Wq, np.float32),
                           np.asarray(Wk, np.float32), np.asarray(Wv, np.float32))
    res = bass_utils.run_bass_kernel_spmd(
        nc, in_maps, core_ids=list(range(8)),
        trace=_trace, tmpdir=_tracedir,
    )
    out = postprocess(res.results)
    if _trace:
        return out, res
    return out


# revision 18
# speedup vs baseline: 2.0361x; 1.0807x over previous
"""Trainium2 Bass kernel for single-head causal attention.

Problem: x[B=4,T=2048,C=1024] -> q,k,v = x@Wq/Wk/Wv [T,64] -> causal softmax(q k^T/sqrt(C)) @ v.

Sharding: 8 cores = 4 batches x 2 query-interleavings. Core r of a batch owns
the 8 INTERLEAVED query blocks g === r (mod 2) (128 rows each), which balances
causal work across the pair (each core gets ~half the attention area).

SPMD-uniform trick: the time axis of each core's x^T copy is permuted so the
core's OWN blocks come first (columns 0-1023, ascending global index), the
other 8 blocks after (ascending). Then the block-causal structure is
identical on every core:
  - k-slot t=0..7  (own blocks, global 2t+r): S over q cols [128t, 1024);
    the leading 128x128 block is the diagonal -> multiplied by a constant
    triangular mask.
  - k-slot t=8..15 (other blocks, global 2(t-8)+1-r): S over q cols
    [128(t-8), 1024); the leading block differs only by DATA: an all-ones
    (r=1: k-block just below the diagonal -> keep) or all-zeros (r=0:
    k-block just above -> drop) multiplier per core.
Softmax normalization is fused into the AV matmul by appending a ones column
to V (output row 64 = sum of exp); division happens host-side on gather.

All matmuls stream bf16 (fp32 matmul is 4 cycles/row). Projections run
chunk-major so they pipeline with the x^T DMA; the own half is loaded and
projected first so S/exp start as early as possible. V is transposed to
natural layout with two batched DMA-xbar transposes.
"""

import numpy as np
import ml_dtypes

B, T, C, H = 4, 2048, 1024, 64
TQ = 1024          # queries per core
NT = 2048          # kv length per core
NCH = C // 128     # 8 contraction chunks
NKT = NT // 128    # 16 k-slots
SCALE = 1.0 / 32.0  # 1/sqrt(C)
VSTRIDE = 80       # bf16 cols per v' slot (64 v + 1 ones + pad, 32B-aligned)

_prog_cache = {}


def _build_program():
    import concourse.mybir as mybir
    from concourse import bacc
    from concourse.tile import TileContext

    fp32 = mybir.dt.float32
    bf16 = mybir.dt.bfloat16
    Exp = mybir.ActivationFunctionType.Exp
    Copy = mybir.ActivationFunctionType.Copy

    nc = bacc.Bacc("TRN2", target_bir_lowering=False, debug=False)

    # xt is pre-tiled host-side: row-block g = SBUF pair-tile g, so every DMA
    # reads 4KB-contiguous lines per partition at full HBM rate.
    xt_d = nc.dram_tensor("xt", [C, NT], bf16, kind="ExternalInput")
    wcat_d = nc.dram_tensor("wcat", [C, 192], bf16, kind="ExternalInput")
    trig_d = nc.dram_tensor("trig", [128, 256], bf16, kind="ExternalInput")
    out_d = nc.dram_tensor("outT", [H + 1, TQ], bf16, kind="ExternalOutput")

    with TileContext(nc) as tc:
        with (
            tc.tile_pool(name="xtp", bufs=1) as xt_pool,
            tc.tile_pool(name="cst", bufs=1) as cst,
            tc.tile_pool(name="prj", bufs=1) as prj,
            tc.tile_pool(name="ptp", bufs=6) as ptp,
            tc.tile_pool(name="pqk", bufs=2, space="PSUM") as pqk,
            tc.tile_pool(name="pvo", bufs=2, space="PSUM") as pvo,
            tc.tile_pool(name="pss", bufs=2, space="PSUM") as pss,
        ):
            # const DMAs on the scalar queue (idle until exp starts)
            wcat_sb = cst.tile([128, NCH, 192], bf16, tag="wcat")
            nc.scalar.dma_start(out=wcat_sb[:], in_=wcat_d.rearrange("(o p) m -> p o m", p=128))
            trig_sb = cst.tile([128, 256], bf16, tag="trig")
            nc.scalar.dma_start(out=trig_sb[:], in_=trig_d[:])

            # x^T halves, 2 chunks per DMA (cols of pair tile: chunk 2p | 2p+1)
            xo_sb, xr_sb = [], []
            for half, lst in ((0, xo_sb), (1, xr_sb)):
                for p in range(4):
                    g = 4 * half + p
                    t = xt_pool.tile([128, 2048], bf16, tag=f"x{half}{p}")
                    nc.sync.dma_start(
                        out=t[:], in_=xt_d[128 * g: 128 * (g + 1), :])
                    lst.append(t)

            def xch(c):
                """rhs AP for contraction chunk c: [128, 1024] time cols."""
                src = xo_sb if c < 8 else xr_sb
                cc = c % 8
                return src[cc // 2][:, 1024 * (cc % 2): 1024 * (cc % 2) + 1024]

            # persistent projection outputs (bf16 so S/AV stream at full rate)
            qT_sb = prj.tile([64, TQ], bf16, tag="qT")
            kT_sb = prj.tile([64, NT], bf16, tag="kT")
            vT_sb = prj.tile([64, NT], bf16, tag="vT")
            vp_sb = prj.tile([128, NKT * VSTRIDE], bf16, tag="vp")
            o_sb = prj.tile([H + 1, TQ], bf16, tag="osb")

            vp3 = vp_sb.rearrange("p (t c) -> p t c", c=VSTRIDE)
            nc.gpsimd.memset(vp3[:, :, 64:65], 1.0)

            # ---- PE warmup on memset data: keep the HAM activity window
            # busy from engine-init so pass 1 runs at full clock ----
            ws_sb = prj.tile([128, 512], bf16, tag="ws")
            wsc_sb = prj.tile([64, 64], bf16, tag="wsc")
            nc.vector.memset(ws_sb[:], 0.125)
            w_ps = pss.tile([128, 512], fp32, tag="s")
            for _ in range(5):
                nc.tensor.matmul(w_ps[:], ws_sb[:, 0:128], ws_sb[:],
                                 start=True, stop=True)
            nc.vector.tensor_copy(out=wsc_sb[:], in_=w_ps[0:64, 0:64])

            # ---- pass 1: own half (q, k, v), chunk-major ----
            qk_ps = [pqk.tile([128, 512], fp32, tag="qk", name=f"qk{h}")
                     for h in range(2)]
            v_ps = [pvo.tile([64, 512], fp32, tag="vo", name=f"pv{h}",
                             padded_shape=[128, 512])
                    for h in range(2)]
            for c in range(NCH):
                for h in range(2):
                    nc.tensor.matmul(
                        qk_ps[h][:], wcat_sb[:, c, 0:128],
                        xch(c)[:, 512 * h: 512 * (h + 1)],
                        start=(c == 0), stop=(c == NCH - 1),
                    )
                for h in range(2):
                    nc.tensor.matmul(
                        v_ps[h][:], wcat_sb[:, c, 128:192],
                        xch(c)[:, 512 * h: 512 * (h + 1)],
                        start=(c == 0), stop=(c == NCH - 1),
                    )
            # q on DVE, k on ACT concurrently (S(0) critical path), then v
            for h in range(2):
                sl = slice(512 * h, 512 * (h + 1))
                nc.vector.tensor_copy(out=qT_sb[:, sl], in_=qk_ps[h][0:64, :])
                nc.scalar.activation(kT_sb[:, sl], qk_ps[h][64:128, :], Copy)
            for h in range(2):
                sl = slice(512 * h, 512 * (h + 1))
                nc.vector.tensor_copy(out=vT_sb[:, sl], in_=v_ps[h][:])
                # own-half v -> natural layout (batched xbar transposes)
                nc.sync.dma_start_transpose(
                    out=vp3[:, 4 * h: 4 * (h + 1), 0:64], in_=vT_sb[:, sl])

            # ---- pass 2: other half (k, v packed), chunk-major ----
            kv_ps = [pqk.tile([128, 512], fp32, tag="qk", name=f"kv{h}")
                     for h in range(2)]

            def emit_pass2(cs):
                for c in cs:
                    for h in range(2):
                        nc.tensor.matmul(
                            kv_ps[h][:], wcat_sb[:, c, 64:192],
                            xch(8 + c)[:, 512 * h: 512 * (h + 1)],
                            start=(c == 0), stop=(c == NCH - 1),
                        )

            def emit_pass2_tail():
                for h in range(2):
                    sl = slice(1024 + 512 * h, 1536 + 512 * h)
                    nc.vector.tensor_copy(out=kT_sb[:, sl], in_=kv_ps[h][0:64, :])
                    nc.vector.tensor_copy(out=vT_sb[:, sl], in_=kv_ps[h][64:128, :])
                    nc.sync.dma_start_transpose(
                        out=vp3[:, 8 + 4 * h: 12 + 4 * h, 0:64], in_=vT_sb[:, sl])

            # ---- S / exp / AV phase ----
            o_ps = [pvo.tile([H + 1, 512], fp32, tag="vo", name=f"o{h}",
                             padded_shape=[128, 512])
                    for h in range(2)]
            pt_tiles = {}

            def emit_S(t):
                a0 = 128 * (t % 8)
                s = pss.tile([128, 1024], fp32, tag="s")
                if a0 < 512:
                    nc.tensor.matmul(
                        s[:, a0:512], kT_sb[:, 128 * t: 128 * (t + 1)],
                        qT_sb[:, a0:512], start=True, stop=True,
                    )
                nc.tensor.matmul(
                    s[:, 512:1024], kT_sb[:, 128 * t: 128 * (t + 1)],
                    qT_sb[:, 512:1024], start=True, stop=True,
                )
                pt = ptp.tile([128, 1024], bf16, tag="pt")
                nc.scalar.activation(pt[:, a0:1024], s[:, a0:1024], Exp, scale=SCALE)
                msk = trig_sb[:, 0:128] if t < 8 else trig_sb[:, 128:256]
                nc.vector.tensor_mul(
                    pt[:, a0:a0 + 128], pt[:, a0:a0 + 128], msk)
                pt_tiles[t] = pt

            def emit_AV(t):
                a0 = 128 * (t % 8)
                pt = pt_tiles.pop(t)
                if a0 < 512:
                    nc.tensor.matmul(
                        o_ps[0][:, a0:512],
                        vp3[:, t, 0:65], pt[:, a0:512],
                        start=(t == 0), stop=(t == 11),
                        skip_group_check=True,
                    )
                b0 = max(a0, 512)
                nc.tensor.matmul(
                    o_ps[1][:, b0 - 512: 512],
                    vp3[:, t, 0:65], pt[:, b0:1024],
                    start=(t == 0), stop=(t == NKT - 1),
                    skip_group_check=True,
                )

            # interleave: pass2 tracks its DMA; S fills the PE between;
            # AV lags S by 3 so exp+mask have time to land.
            emit_pass2(range(0, 4))
            emit_S(0)
            emit_S(1)
            emit_S(2)
            emit_S(3)
            emit_AV(0)
            emit_pass2(range(4, 8))
            emit_pass2_tail()
            emit_S(4)
            emit_AV(1)
            emit_S(5)
            emit_AV(2)
            emit_S(6)
            emit_AV(3)
            emit_S(7)
            emit_AV(4)
            for t in range(8, NKT):
                emit_S(t)
                emit_AV(t - 3)
                if t - 3 == 11:  # o_ps[0] complete; drain it early
                    nc.vector.tensor_copy(out=o_sb[:, 0:512], in_=o_ps[0][:])
                    nc.sync.dma_start(out=out_d[:, 0:512], in_=o_sb[:, 0:512])
            for t in range(NKT - 3, NKT):
                emit_AV(t)
            nc.vector.tensor_copy(out=o_sb[:, 512:1024], in_=o_ps[1][:])
            nc.sync.dma_start(out=out_d[:, 512:1024], in_=o_sb[:, 512:1024])

    nc.finalize()
    return nc


def _get_program():
    if "nc" not in _prog_cache:
        _prog_cache["nc"] = _build_program()
    return _prog_cache["nc"]


def make_in_maps(x, Wq, Wk, Wv):
    bf16 = ml_dtypes.bfloat16
    wcat = np.concatenate([Wq, Wk, Wv], axis=1).astype(bf16)  # [C, 192]
    tri = np.triu(np.ones((128, 128), np.float32))  # tri[k,q]=1 iff q>=k
    in_maps = []
    for core in range(8):
        b, r = core // 2, core % 2
        xb = np.asarray(x[b]).reshape(16, 128, C)
        own = xb[r::2].reshape(TQ, C)
        other = xb[1 - r::2].reshape(TQ, C)
        xtT = np.concatenate([own, other], axis=0).T  # [C, 2048]
        # pre-tile for the DMA: row-block g=4*half+p holds chunks 2p|2p+1 of
        # that half so each [128,2048] SBUF tile is DRAM-contiguous
        xt = (xtT.reshape(4, 2, 128, 2, 1024)
              .transpose(3, 0, 2, 1, 4).reshape(C, 2048))
        gate = np.full((128, 128), float(r), np.float32)
        trig = np.concatenate([tri, gate], axis=1).astype(bf16)  # [128, 256]
        in_maps.append({
            "xt": np.ascontiguousarray(xt).astype(bf16),
            "wcat": wcat,
            "trig": trig,
        })
    return in_maps


def postprocess(results):
    out = np.empty((B, T, H), np.float32)
    for core in range(8):
        b, r = core // 2, core % 2
        oT = np.asarray(results[core]["outT"], np.float32)  # [65, 1024]
        o = (oT[:H] / oT[H:H + 1]).T  # [1024, 64] local q order
        for i in range(8):
            g = 2 * i + r
            out[b, 128 * g: 128 * (g + 1)] = o[128 * i: 128 * (i + 1)]
    return out


def kernel(x, mask, Wq, Wk, Wv, _trace=False, _tracedir=None):
    from concourse import bass_utils

    nc = _get_program()
    in_maps = make_in_maps(np.asarray(x, np.float32), np.asarray(Wq, np.float32),
                           np.asarray(Wk, np.float32), np.asarray(Wv, np.float32))
    res = bass_utils.run_bass_kernel_spmd(
        nc, in_maps, core_ids=list(range(8)),
        trace=_trace, tmpdir=_tracedir,
    )
    out = postprocess(res.results)
    if _trace:
        return out, res
    return out
